# revision 62
# baseline (speedup 1.0000x reference)
"""AWQ W4 grouped-dequant matmul on 8 Trainium2 cores.

y = (x / s) @ (w_q * scales).reshape(OUT, IN).T + bias

Column-parallel sharding: each core owns OUT/8 = 1376 output channels
(padded to 1408 = 11*128), x is replicated. Per core the kernel computes
y_shard^T [1408, 2048] = W'[1408, 4096] @ x_bf16[4096, 2048] where the
smoothing division is folded into the weights: W' = (w_q * scales) / s.

On-chip work per core:
  - Dequant once (resident 11.5 MB bf16): wd = (bf16(w_q) * (1/s)_k) *
    scales, fused in one VectorE scalar_tensor_tensor; scales reach all
    128 partitions via a GpSimd partition_broadcast.
  - PE bf16 matmuls accumulate K=4096 in PSUM fp32 (32 x K=128).
  - PSUM evict on VectorE fuses the bias add, writing a resident bf16
    y^T tile; plain DMAs stream it out.

The toolchain permits AT MOST ONE semaphore wait per instruction
(including DMAs). The structure guarantees this: every DMA target is
either write-once or recycles on the same HWDGE lane (8 x-loads per
chunk, pool bufs=3 -> multiples of 8 between same-region writes), tiny
"absorber" ops pre-observe stale ticks (lane absorbers for the PE WAR
on recycled x tiles; zero-matmuls open PSUM groups to take the
slot-release wait; 1-element DVE/Pool touches split double-waits).

Host side does only layout/dtype moves: transpose, pad, shard, bf16
casts (w_q ints are exact in bf16).
"""

import os
from contextlib import ExitStack

import numpy as np

# ---- problem constants (hardcoded per contract) ----
OUT, N_GROUPS, GROUP = 11008, 32, 128
IN = N_GROUPS * GROUP  # 4096
TOKENS = 2048
N_CORES = 8
P = 128
O_SHARD = OUT // N_CORES  # 1376
O_PAD = 1408  # 11 * 128
WPAD = O_PAD + 8  # zero pad cols: PE absorbs W-load ticks via them
OT = O_PAD // P  # 11 o-tiles
KT = IN // P  # 32 k-tiles (== quant groups, GROUP == P)
TCH = 256  # tokens per moving chunk
NT = TOKENS // TCH  # 8
NLANES = 8  # HWDGE lanes; x-loads per chunk must be a multiple
KPD = KT // NLANES  # k-tiles per x-load DMA (4)

LAST = {}  # exec_time_ns etc. for the local test harness

_NC_CACHE = {}


def _build_nc():
    import concourse.bass as bass
    from concourse import mybir
    from contextlib import ExitStack

    f32 = mybir.dt.float32
    bf16 = mybir.dt.bfloat16

    nc = bass.Bass()
    xT = nc.declare_dram_parameter("xT", [IN, TOKENS], bf16, isOutput=False)
    wT = nc.declare_dram_parameter("wT", [IN, O_PAD], bf16, isOutput=False)
    sc_repl = nc.declare_dram_parameter(
        "sc_repl", [N_GROUPS, P, O_PAD], bf16, isOutput=False
    )
    s_cols = nc.declare_dram_parameter("s_cols", [P, KT], f32, isOutput=False)
    bias_cols = nc.declare_dram_parameter("bias_cols", [P, OT], f32, isOutput=False)
    yT = nc.declare_dram_parameter("yT", [O_PAD, TOKENS], bf16, isOutput=True)

    NB = 6  # psum banks in rotation
    XB = 3  # x chunk buffers
    SCB = 4  # scales staging buffers
    NG = NT * OT  # 88 psum groups

    with ExitStack() as ctx:
        w_all = ctx.enter_context(nc.sbuf_tensor("w_all", [P, KT * O_PAD], bf16))
        xn_all = ctx.enter_context(nc.sbuf_tensor("xn_all", [P, XB * KT * TCH], bf16))
        y_sb = ctx.enter_context(nc.sbuf_tensor("y_sb", [P, OT * TOKENS], bf16))
        scb_all = ctx.enter_context(nc.sbuf_tensor("scb_all", [P, SCB * O_PAD], bf16))
        s_sb = ctx.enter_context(nc.sbuf_tensor("s_sb", [P, KT], f32))
        inv_s = ctx.enter_context(nc.sbuf_tensor("inv_s", [P, KT], f32))
        bias_sb = ctx.enter_context(nc.sbuf_tensor("bias_sb", [P, OT], f32))
        ps = [
            ctx.enter_context(nc.psum_tensor(f"ps{i}", [P, TCH], f32)) for i in range(NB)
        ]
        s_c = ctx.enter_context(nc.semaphore("s_c"))
        s_q = [
            ctx.enter_context(nc.semaphore(f"s_q{j}")) for j in range(8)
        ]
        s_scq = [
            ctx.enter_context(nc.semaphore(f"s_scq{j}")) for j in range(SCB)
        ]
        s_pe = ctx.enter_context(nc.semaphore("s_pe"))
        s_dve = ctx.enter_context(nc.semaphore("s_dve"))
        block = ctx.enter_context(nc.Block())

        def wd(kc):
            return w_all[:, kc * O_PAD : (kc + 1) * O_PAD]

        def xreg(tt, kc):
            b = tt % XB
            o = (b * KT + kc) * TCH
            return xn_all[:, o : o + TCH]

        def yreg(tt, ot):
            o = ot * TOKENS + tt * TCH
            return y_sb[:, o : o + TCH]

        # chained counting scheme on s_q[j]: wd loads contribute 4 counts
        # per lane, x loads 4 per chunk per lane, y stores 11 per lane;
        # every DMA waits for its lane predecessor so increments are
        # ordered (8 transfers in flight across lanes).
        def wd_idx(kc):
            return kc // 8  # 0..3

        def xn_idx(tt, kc):
            return 4 + 4 * tt + kc // 8

        def y_idx(g):
            return 4 + 4 * NT + g // 8

        @block.sync
        def _(sync):
            sync.dma_start(out=s_sb[:, :], in_=s_cols[:, :]).then_inc(s_c, 16)
            sync.dma_start(out=bias_sb[:, :], in_=bias_cols[:, :]).then_inc(
                s_c, 16
            )
            for kc in range(KT):
                j = kc % 8
                if wd_idx(kc) > 0:
                    sync.wait_ge(s_q[j], 16 * wd_idx(kc))
                sync.dma_start(
                    out=wd(kc), in_=wT[kc * P : (kc + 1) * P, :]
                ).then_inc(s_q[j], 16)
                sj = kc % SCB
                if kc >= SCB:
                    # staging slot reused: wait for its dequant mul (this
                    # also orders the s_scq chain transitively)
                    sync.wait_ge(s_dve, 2 + (kc - SCB))
                sc_slot = scb_all[:, sj * O_PAD : (sj + 1) * O_PAD]
                sync.dma_start(out=sc_slot, in_=sc_repl[kc, :, :]).then_inc(
                    s_scq[sj], 16
                )
            for tt in range(NT):
                t0 = tt * TCH
                for kc in range(KT):
                    j = kc % 8
                    sync.wait_ge(s_q[j], 16 * xn_idx(tt, kc))
                    if tt >= XB:
                        # x buffer reused: wait for its last reader
                        last_rd = ((tt - XB) * OT + OT - 1) * KT + kc + 1
                        sync.wait_ge(s_pe, last_rd)
                    sync.dma_start(
                        out=xreg(tt, kc),
                        in_=xT[kc * P : (kc + 1) * P, t0 : t0 + TCH],
                    ).then_inc(s_q[j], 16)
            for g in range(NG):
                tt, ot = g // OT, g % OT
                j = g % 8
                sync.wait_ge(s_q[j], 16 * y_idx(g))
                sync.wait_ge(s_dve, 34 + g)
                sync.dma_start(
                    out=yT[ot * P : (ot + 1) * P, tt * TCH : (tt + 1) * TCH],
                    in_=yreg(tt, ot),
                ).then_inc(s_q[j], 16)
            for j in range(8):
                sync.wait_ge(s_q[j], 16 * (4 + 4 * NT + NG // 8))

        @block.vector
        def _(vector):
            vector.wait_ge(s_c, 32)
            nc.vector.reciprocal(out=inv_s[:, :], in_=s_sb[:, :]).then_inc(
                s_dve, 1
            )
            vector.wait_ge(s_dve, 1)  # recip retired before muls read inv_s
            for kc in range(KT):
                vector.wait_ge(s_q[kc % 8], 16 * (wd_idx(kc) + 1))
                vector.wait_ge(s_scq[kc % SCB], 16 * (kc // SCB + 1))
                sc_slot = scb_all[
                    :, (kc % SCB) * O_PAD : (kc % SCB + 1) * O_PAD
                ]
                nc.vector.scalar_tensor_tensor(
                    wd(kc),
                    wd(kc),
                    inv_s[:, kc : kc + 1],
                    sc_slot,
                    mybir.AluOpType.mult,
                    mybir.AluOpType.mult,
                ).then_inc(s_dve, 1)
            for g in range(NG):
                tt, ot = g // OT, g % OT
                vector.wait_ge(s_pe, 32 * (g + 1))
                nc.vector.tensor_scalar_add(
                    yreg(tt, ot), ps[g % NB][:, :], bias_sb[:, ot : ot + 1]
                ).then_inc(s_dve, 1)

        @block.tensor
        def _(tensor):
            for g in range(NG):
                tt, ot = g // OT, g % OT
                if g >= NB:
                    # psum bank reused: wait for its evict
                    tensor.wait_ge(s_dve, 34 + g - NB)
                for kc in range(KT):
                    if g == 0:
                        tensor.wait_ge(s_dve, 2 + kc)  # dequant of wd(kc)
                    if ot == 0:
                        tensor.wait_ge(
                            s_q[kc % 8], 16 * (xn_idx(tt, kc) + 1)
                        )
                    nc.tensor.matmul(
                        ps[g % NB][:, :],
                        wd(kc)[:, ot * P : (ot + 1) * P],
                        xreg(tt, kc),
                        start=(kc == 0),
                        stop=(kc == KT - 1),
                    ).then_inc(s_pe, 1)
    return nc


def get_nc():
    if "nc" not in _NC_CACHE:
        _NC_CACHE["nc"] = _build_nc()
    return _NC_CACHE["nc"]


def _prep_inputs(x, w_q, scales, s, bias):
    import ml_dtypes

    bf16 = ml_dtypes.bfloat16
    x = np.asarray(x, dtype=np.float32)
    w_q = np.asarray(w_q)
    scales = np.asarray(scales, dtype=np.float32)
    s = np.asarray(s, dtype=np.float32)
    bias = np.asarray(bias, dtype=np.float32)

    pad = O_PAD - O_SHARD  # 32 rows of zero-padding per shard
    # weights: int in [-7,7] -> bf16 exact
    w = w_q.reshape(OUT, IN).astype(bf16)
    sc = scales.reshape(OUT, N_GROUPS)  # f32

    xT = np.ascontiguousarray(x.T.astype(bf16))  # [IN, TOKENS] bf16
    s_cols = np.ascontiguousarray(s.reshape(KT, P).T)  # [128, 32] f32

    in_maps = []
    for c in range(N_CORES):
        lo, hi = c * O_SHARD, (c + 1) * O_SHARD
        w_c = np.pad(w[lo:hi], ((0, pad), (0, 0)))  # [O_PAD, IN]
        sc_c = np.pad(sc[lo:hi], ((0, pad), (0, 0)))  # [O_PAD, 32]
        b_c = np.pad(bias[lo:hi], (0, pad))  # [O_PAD]
        in_maps.append(
            {
                "xT": xT,
                "wT": np.ascontiguousarray(w_c.T),  # [IN, O_PAD] bf16
                "sc_repl": np.ascontiguousarray(
                    np.broadcast_to(
                        sc_c.T.astype(bf16)[:, None, :], (N_GROUPS, P, O_PAD)
                    )
                ),  # [32, 128, O_PAD] bf16
                "s_cols": s_cols,
                "bias_cols": np.ascontiguousarray(
                    b_c.reshape(OT, P).T
                ),  # [128, 11] f32
            }
        )
    return in_maps


def _install_profile_shim():
    """Provide antenv.axon_hooks (NTFF profiling via libaxon ctypes) when
    the container image lacks it. Only used for local perf iteration."""
    import contextlib
    import ctypes
    import sys
    import types

    if "antenv.axon_hooks" in sys.modules:
        return
    so_path = "/opt/axon/libaxon_pjrt.so"
    try:
        lib = ctypes.CDLL(so_path)
    except OSError:
        return
    if not hasattr(lib, "axon_start_nrt_profile"):
        return
    lib.axon_start_nrt_profile.argtypes = [
        ctypes.POINTER(ctypes.c_int64),
        ctypes.c_size_t,
    ]
    lib.axon_start_nrt_profile.restype = ctypes.c_int64
    lib.axon_stop_nrt_profile.argtypes = [ctypes.c_char_p]
    lib.axon_stop_nrt_profile.restype = ctypes.c_int64

    @contextlib.contextmanager
    def _hook(output_dir, device_ids):
        import jax

        jax.devices()
        if device_ids:
            ids = (ctypes.c_int64 * len(device_ids))(*device_ids)
            rc = lib.axon_start_nrt_profile(ids, len(device_ids))
        else:
            rc = lib.axon_start_nrt_profile(None, 0)
        if rc != 0:
            raise RuntimeError(f"axon_start_nrt_profile rc={rc}")
        try:
            yield
        finally:
            n = lib.axon_stop_nrt_profile(str(output_dir).encode())
            print(f"profile: {n} file(s) written to {output_dir}", file=sys.stderr)

    mod = types.ModuleType("antenv.axon_hooks")
    mod.get_axon_ntff_profile_hook = lambda: _hook
    mod.set_axon_ntff_profile_hook = lambda h: None
    sys.modules["antenv.axon_hooks"] = mod


def kernel(x, w_q, scales, s, bias):
    import sys

    if "/opt/trn_rl_repo" not in sys.path:
        sys.path.insert(0, "/opt/trn_rl_repo")
    import concourse.bass_utils as bass_utils
    from concourse.bass_utils import run_bass_kernel_spmd

    orig_dtype = np.asarray(x).dtype
    in_maps = _prep_inputs(x, w_q, scales, s, bias)
    nc = get_nc()

    trace = bool(os.environ.get("AWQ_TRACE"))
    kwargs = {}
    if trace:
        _install_profile_shim()
        bass_utils.upload_artifacts = lambda d: d  # zero-egress container
        tmpdir = os.environ.get("AWQ_TRACE_DIR")
        if tmpdir:
            os.makedirs(tmpdir, exist_ok=True)
            kwargs["tmpdir"] = tmpdir
    res = run_bass_kernel_spmd(
        nc,
        in_maps,
        core_ids=list(range(N_CORES)),
        trace=trace,
        **kwargs,
    )
    LAST["exec_time_ns"] = res.exec_time_ns
    LAST["results"] = res

    yT_full = np.concatenate(
        [np.asarray(res.results[c]["yT"], dtype=np.float32) for c in range(N_CORES)],
        axis=0,
    )  # [8*1408, 2048] f32
    y = np.ascontiguousarray(
        yT_full.reshape(N_CORES, O_PAD, TOKENS)[:, :O_SHARD, :]
        .reshape(OUT, TOKENS)
        .T
    )
    return y.astype(orig_dtype)
